# revision 13
# baseline (speedup 1.0000x reference)
"""CenterHead inference kernel for Trainium2 (8 NeuronCores, SPMD).

Strategy
--------
Spatially shard the 360-row BEV map into 8 H-shards of 45 rows. Each core
receives its own zero-padded x slab (51 rows incl. 3-row halo each side) plus
the full (tiny) weight set, and computes its shard of all 12 output channels
(hm logits x3, iou, ct x2, cz, dim x3, rot x2) with exact-fp32 matmuls:

  shared 3x3 conv (128->64) -> per branch-pair 3x3 conv (64->128, two
  branches packed into the M dim, 3x3 taps pair-packed into K via a
  row-shifted duplicate of the shared feature map) -> 3x3 conv to the
  per-branch output channels (branch pair block-diagonal in K).

Convs are computed as flat-image shift-and-accumulate matmuls over the
W-padded (362-wide) row-major layout; junk values produced at the pad
columns are re-zeroed between stages and stripped at the end.

The decode (sigmoid/top-k/gather/box math/argsort/NMS) runs on the host on
the gathered 12x360x360 maps: it is O(K^2)=250k scalar ops vs the ~70 GFLOP
conv stage, and keeping it in fp64-free numpy f32 reproduces the reference
bit-layout closely.
"""

import os

import numpy as np

B, CIN, CSH, H, W = 1, 128, 64, 360, 360
NUM_CLS = 3
K = 500
SCORE_TH = 0.1
NMS_TH = 0.7
VOXEL = 0.32
STRIDE = 1
PC_MIN_X, PC_MIN_Y = -57.6, -57.6
POST_RANGE = np.array([-61.2, -61.2, -10.0, 61.2, 61.2, 10.0], np.float32)
IOU_RECT = np.array([0.68, 0.71, 0.65], np.float32)

NCORES = 8
HS = H // NCORES           # 45 out rows per core
WP = W + 2                 # 362 padded width
XROWS = HS + 6             # 51 x rows per core (3-row halo each side)
XN = XROWS * WP            # 18462
SHROWS = HS + 4            # 49 shared rows ([-2, +2) halo)
SHN = SHROWS * WP          # 17738
HROWS = HS + 2             # 47 hidden rows ([-1, +1) halo)
HN = HROWS * WP            # 17014
ON = HS * WP               # 16290
MMC = 512                  # matmul free-dim chunk

# branch pairs: precision-critical (hm, iou) first; couts per branch
PAIRS = [("hm", "iou"), ("ct", "cz"), ("dim", "rot")]
COUT = {"hm": 3, "ct": 2, "cz": 1, "dim": 3, "rot": 2, "iou": 1}
PAIR_M = [COUT[a] + COUT[b] for a, b in PAIRS]        # [4, 3, 5]
PAIR_ROW0 = [0, 4, 7]                                 # out_maps row offsets
TAPS = [(dy, dx) for dy in range(3) for dx in range(3)]

_CACHE = {}
_MM_F32R = bool(os.environ.get("KERNEL_F32R"))  # fp32r matmuls (4x faster, reduced precision)


def _build_program(reps=1):
    import concourse.bass as bass
    import concourse.tile as tile
    from concourse import bacc, mybir

    f32 = mybir.dt.float32
    nc = bacc.Bacc(
        "TRN2",
        debug=False,
        enable_asserts=False,
        target_bir_lowering=False,
        num_devices=NCORES,
    )

    # --- DRAM I/O ---
    x_d = nc.dram_tensor("xc", [128, XN], f32, kind="ExternalInput").ap()
    msk_d = nc.dram_tensor("msk", [128, 51 + HROWS], f32, kind="ExternalInput").ap()
    wsh_d = nc.dram_tensor("wsh", [128, 9 * 64], f32, kind="ExternalInput").ap()
    ssh_d = nc.dram_tensor("ssh", [64, 1], f32, kind="ExternalInput").ap()
    bsh_d = nc.dram_tensor("bsh", [64, 1], f32, kind="ExternalInput").ap()
    w0d02_d, w0d1_d, s0_d, b0_d, w1_d = [], [], [], [], []
    for p, m in enumerate(PAIR_M):
        w0d02_d.append(nc.dram_tensor(f"w0d02_{p}", [128, 3 * 128], f32, kind="ExternalInput").ap())
        w0d1_d.append(nc.dram_tensor(f"w0d1_{p}", [64, 3 * 128], f32, kind="ExternalInput").ap())
        s0_d.append(nc.dram_tensor(f"s0_{p}", [128, 1], f32, kind="ExternalInput").ap())
        b0_d.append(nc.dram_tensor(f"b0_{p}", [128, 1], f32, kind="ExternalInput").ap())
        w1_d.append(nc.dram_tensor(f"w1_{p}", [128, 3 * 96], f32, kind="ExternalInput").ap())
    out_d = nc.dram_tensor("out_maps", [12, ON], f32, kind="ExternalOutput").ap()

    with tile.TileContext(nc) as tc:
        for _ in range(reps):
            _emit(tc, nc, bass, mybir, x_d, msk_d, wsh_d, ssh_d, bsh_d,
                  w0d02_d, w0d1_d, s0_d, b0_d, w1_d, out_d)

    nc.compile()
    return nc


def _emit(tc, nc, bass, mybir, x_d, msk_d, wsh_d, ssh_d, bsh_d,
          w0d02_d, w0d1_d, s0_d, b0_d, w1_d, out_d):
    from contextlib import ExitStack

    f32 = mybir.dt.float32
    AF = mybir.ActivationFunctionType
    mmdt = mybir.dt.float32r if _MM_F32R else f32

    def mm(out, lhsT, rhs, start, stop):
        nc.tensor.matmul(out, lhsT.bitcast(mmdt), rhs.bitcast(mmdt),
                         start=start, stop=stop)

    with ExitStack() as ctx:
        constp = ctx.enter_context(tc.tile_pool(name="const", bufs=1))

        def load_const(ap_d, shape, tag):
            t = constp.tile(shape, f32, tag=tag)
            nc.sync.dma_start(t[:], ap_d[:])
            return t

        wsh = load_const(wsh_d, [128, 9 * 64], "wsh")
        mskt = load_const(msk_d, [128, 51 + HROWS], "msk")
        ssh = load_const(ssh_d, [64, 1], "ssh")
        bsh = load_const(bsh_d, [64, 1], "bsh")
        w0d02 = [load_const(w0d02_d[p], [128, 3 * 128], f"w0d02_{p}") for p in range(3)]
        w0d1 = [load_const(w0d1_d[p], [64, 3 * 128], f"w0d1_{p}") for p in range(3)]
        s0 = [load_const(s0_d[p], [128, 1], f"s0_{p}") for p in range(3)]
        b0 = [load_const(b0_d[p], [128, 1], f"b0_{p}") for p in range(3)]
        w1 = [load_const(w1_d[p], [128, 3 * 96], f"w1_{p}") for p in range(3)]

        # shared feature map, duplicated layout:
        #   partitions 0:64  row j (of 51)   = shared local row j-1 (rows 0,50 zero)
        #   partitions 64:128 row j          = lower row j+2
        shp = ctx.enter_context(tc.tile_pool(name="shp", bufs=1))
        sh = shp.tile([128, 51 * WP], f32)

        # ---- phase A/B: x DMA + shared conv ----
        with tc.tile_pool(name="xp", bufs=1) as xp, \
             tc.tile_pool(name="psA", bufs=4, space="PSUM") as psA_pool:
            xt = xp.tile([128, XN + 2], f32)   # 1-elem guards both ends
            nc.vector.memset(xt[:, 0:1], 0.0)
            nc.vector.memset(xt[:, XN + 1:XN + 2], 0.0)
            # zero regions of sh that are never written but are read
            nc.vector.memset(sh[0:64, 0:WP], 0.0)                       # lower row 0
            nc.vector.memset(sh[0:64, 50 * WP:51 * WP], 0.0)            # lower row 50
            nc.vector.memset(sh[64:128, 48 * WP:51 * WP], 0.0)          # upper rows 48..50

            # x DMA in 6 row-chunks for overlap with compute
            row_edges = [0, 9, 18, 27, 36, 44, 51]
            for r0, r1 in zip(row_edges[:-1], row_edges[1:]):
                nc.sync.dma_start(xt[:, 1 + r0 * WP:1 + r1 * WP],
                                  x_d[:, r0 * WP:r1 * WP])

            for g0 in range(0, SHN, MMC):
                n = min(MMC, SHN - g0)
                ps = psA_pool.tile([64, MMC], f32, tag="psA")
                for t, (dy, dx) in enumerate(TAPS):
                    o = 1 + g0 + dy * WP + dx - 1
                    mm(ps[:, :n], wsh[:, 64 * t:64 * t + 64],
                       xt[:, o:o + n], start=(t == 0), stop=(t == 8))
                # lower copy: relu(ps*s+b) -> sh[0:64] at flat g0+WP
                nc.scalar.activation(sh[0:64, WP + g0:WP + g0 + n], ps[:, :n],
                                     AF.Relu, bias=bsh[:, 0:1], scale=ssh[:, 0:1])
                # upper copy: same values shifted down 2 rows (skip local row 0)
                s_off = max(0, WP - g0)
                if n > s_off:
                    nc.scalar.activation(
                        sh[64:128, g0 - WP + s_off:g0 - WP + n], ps[:, s_off:n],
                        AF.Relu, bias=bsh[:, 0:1], scale=ssh[:, 0:1])

        # re-zero the W-pad columns (flat-conv wrote junk there)
        sh3 = sh.rearrange("p (r c) -> p r c", c=WP)
        nc.vector.memset(sh3[0:64, 1:50, 0:1], 0.0)
        nc.vector.memset(sh3[0:64, 1:50, 361:362], 0.0)
        nc.vector.memset(sh3[64:128, 0:48, 0:1], 0.0)
        nc.vector.memset(sh3[64:128, 0:48, 361:362], 0.0)
        # zero the out-of-image halo rows (reference SAME-pad semantics);
        # mask is per-core input data so the SPMD program stays uniform
        msk_sh = mskt[:, 0:51].rearrange("p (r o) -> p r o", o=1).broadcast_to((128, 51, WP))
        nc.vector.tensor_mul(sh3[:, :, :], sh3[:, :, :], msk_sh)

        # ---- phase C/D: branch pairs ----
        with tc.tile_pool(name="hp", bufs=1) as hp, \
             tc.tile_pool(name="bounce", bufs=6) as bouncep:
            for p in range(3):
                m = PAIR_M[p]
                row0 = PAIR_ROW0[p]
                hid = hp.tile([128, HN + 2], f32, tag="hidden")
                nc.vector.memset(hid[:, 0:1], 0.0)
                nc.vector.memset(hid[:, HN + 1:HN + 2], 0.0)

                # conv0: shared(64) -> pair hidden(128); taps (0,dx)+(2,dx)
                # pair-packed in K via the upper copy of sh; (1,dx) K=64.
                psB_pool = tc.alloc_tile_pool(name=f"psB{p}", bufs=4, space="PSUM")
                for f0 in range(0, HN, MMC):
                    n = min(MMC, HN - f0)
                    ps = psB_pool.tile([128, MMC], f32, tag="psB")
                    for i, dx in enumerate(range(3)):
                        o = f0 + WP + dx - 1
                        mm(ps[:, :n], w0d02[p][:, 128 * dx:128 * dx + 128],
                           sh[0:128, o:o + n], start=(i == 0), stop=False)
                    for i, dx in enumerate(range(3)):
                        o = f0 + 2 * WP + dx - 1
                        mm(ps[:, :n], w0d1[p][0:64, 128 * dx:128 * dx + 128],
                           sh[0:64, o:o + n], start=False, stop=(i == 2))
                    nc.scalar.activation(hid[:, 1 + f0:1 + f0 + n], ps[:, :n],
                                         AF.Relu, bias=b0[p][:, 0:1], scale=s0[p][:, 0:1])

                hid3 = hid[:, 1:1 + HN].rearrange("p (r c) -> p r c", c=WP)
                nc.vector.memset(hid3[:, :, 0:1], 0.0)
                nc.vector.memset(hid3[:, :, 361:362], 0.0)
                msk_h = mskt[:, 51:51 + HROWS].rearrange(
                    "p (r o) -> p r o", o=1).broadcast_to((128, HROWS, WP))
                nc.vector.tensor_mul(hid3[:, :, :], hid3[:, :, :], msk_h)
                psB_pool.release()
                psC_pool = tc.alloc_tile_pool(name=f"psC{p}", bufs=2, space="PSUM")

                # conv1 via M=(3 dy)x(m) partial sums: per 4-row hidden
                # chunk, 3 dx-matmuls produce g[(dy,c), hy, x]; the dy row
                # shift is applied afterwards on DVE (2 adds per dy).
                gts = {}
                n_g = (HROWS + 3) // 4            # 12 g chunks (last has 3 rows)
                for k in range(n_g):
                    rows = min(4, HROWS - 4 * k)
                    span = rows * WP
                    gt = psC_pool.tile([96, 4 * WP], f32, tag="psC")
                    gts[k] = gt
                    for sc0 in range(0, span, MMC):
                        ns = min(MMC, span - sc0)
                        for dx in range(3):
                            o = 1 + 4 * k * WP + sc0 + dx - 1
                            mm(gt[:96, sc0:sc0 + ns], w1[p][:, 96 * dx:96 * dx + 96],
                               hid[:, o:o + ns], start=(dx == 0), stop=(dx == 2))
                    # dy-sum for the out chunk that is now fully computable
                    for ok in ([k - 1] if k > 0 else []) + ([k] if k == n_g - 1 else []):
                        o0 = 4 * ok
                        o1 = min(o0 + 4, HS)
                        if o1 <= o0:
                            continue
                        osp = (o1 - o0) * WP
                        bt = bouncep.tile([8, 4 * WP], f32, tag="bounce")
                        ga, gb = gts[ok], gts.get(ok + 1)
                        # dy=0: g rows [o0, o1) == tile ok rows [0, o1-o0)
                        nc.vector.tensor_copy(bt[:m, :osp], ga[0:m, 0:osp])
                        # dy=1: g rows [o0+1, o1+1)
                        a_rows = min(o1 + 1, 4 * ok + 4) - (o0 + 1)
                        if a_rows > 0:
                            nc.vector.tensor_add(bt[:m, 0:a_rows * WP], bt[:m, 0:a_rows * WP],
                                                 ga[32:32 + m, WP:WP + a_rows * WP])
                        b_rows = (o1 + 1) - max(o0 + 1, 4 * ok + 4)
                        if b_rows > 0:
                            nc.vector.tensor_add(bt[:m, a_rows * WP:osp], bt[:m, a_rows * WP:osp],
                                                 gb[32:32 + m, 0:b_rows * WP])
                        # dy=2: g rows [o0+2, o1+2)
                        a_rows2 = min(o1 + 2, 4 * ok + 4) - (o0 + 2)
                        if a_rows2 > 0:
                            nc.vector.tensor_add(bt[:m, 0:a_rows2 * WP], bt[:m, 0:a_rows2 * WP],
                                                 ga[64:64 + m, 2 * WP:2 * WP + a_rows2 * WP])
                        b_rows2 = (o1 + 2) - max(o0 + 2, 4 * ok + 4)
                        if b_rows2 > 0:
                            nc.vector.tensor_add(bt[:m, a_rows2 * WP:osp], bt[:m, a_rows2 * WP:osp],
                                                 gb[64:64 + m, 0:b_rows2 * WP])
                        nc.sync.dma_start(out_d[row0:row0 + m, o0 * WP:o0 * WP + osp],
                                          bt[:m, :osp])
                        gts.pop(ok - 1, None)
                psC_pool.release()


def _pack_weights(inputs):
    f = np.float32
    a = {k: np.asarray(v, f) for k, v in inputs.items()}
    maps = {}
    w_sh = a["w_shared"]  # [64,128,3,3]
    maps["wsh"] = np.ascontiguousarray(np.concatenate(
        [w_sh[:, :, dy, dx].T for dy, dx in TAPS], axis=1))
    maps["ssh"] = a["s_shared"][:, None]
    maps["bsh"] = a["b_shared"][:, None]
    for p, (A, Bn) in enumerate(PAIRS):
        wA0, wB0 = a[f"w_{A}0"], a[f"w_{Bn}0"]

        def pairM(dy, dx):
            return np.concatenate([wA0[:, :, dy, dx].T, wB0[:, :, dy, dx].T], axis=1)

        maps[f"w0d02_{p}"] = np.ascontiguousarray(np.concatenate(
            [np.concatenate([pairM(0, dx), pairM(2, dx)], axis=0) for dx in range(3)],
            axis=1))
        maps[f"w0d1_{p}"] = np.ascontiguousarray(np.concatenate(
            [pairM(1, dx) for dx in range(3)], axis=1))
        maps[f"s0_{p}"] = np.concatenate([a[f"s_{A}0"], a[f"s_{Bn}0"]])[:, None]
        maps[f"b0_{p}"] = np.concatenate([a[f"b_{A}0"], a[f"b_{Bn}0"]])[:, None]
        wA1, wB1 = a[f"w_{A}1"], a[f"w_{Bn}1"]
        cA, cB = COUT[A], COUT[Bn]
        cols = []
        for dx in range(3):
            z = np.zeros((128, 96), f)
            for dy in range(3):
                z[0:64, 32 * dy:32 * dy + cA] = wA1[:, :, dy, dx].T
                z[64:128, 32 * dy + cA:32 * dy + cA + cB] = wB1[:, :, dy, dx].T
            cols.append(z)
        maps[f"w1_{p}"] = np.ascontiguousarray(np.concatenate(cols, axis=1))
    return {k: np.ascontiguousarray(v, f) for k, v in maps.items()}


def _pack_x(x):
    xp = np.zeros((128, H + 6, WP), np.float32)
    xp[:, 3:3 + H, 1:1 + W] = np.asarray(x, np.float32)[0]
    return [np.ascontiguousarray(xp[:, HS * c:HS * c + XROWS, :].reshape(128, XN))
            for c in range(NCORES)]


def _pack_masks():
    msks = []
    for c in range(NCORES):
        m = np.zeros((128, 51 + HROWS), np.float32)
        for j in range(51):
            m[0:64, j] = 1.0 if 0 <= 45 * c + j - 3 <= 359 else 0.0
            m[64:128, j] = 1.0 if 0 <= 45 * c + j - 1 <= 359 else 0.0
        for hy in range(HROWS):
            m[:, 51 + hy] = 1.0 if 0 <= 45 * c - 1 + hy <= 359 else 0.0
        msks.append(m)
    return msks


def _get_runner():
    """Build (once) the jitted 8-core shard_map runner for the Bass program."""
    if "runner" in _CACHE:
        return _CACHE["runner"]
    import jax
    from jax.sharding import Mesh, NamedSharding, PartitionSpec
    from jax.experimental.shard_map import shard_map
    import concourse.mybir as mybir
    from concourse.bass2jax import _bass_exec_p, partition_id_tensor

    if "nc" not in _CACHE:
        _CACHE["nc"] = _build_program()
    nc = _CACHE["nc"]

    partition_name = nc.partition_id_tensor.name if nc.partition_id_tensor else None
    in_names, out_names, out_avals, zero_outs = [], [], [], []
    for alloc in nc.m.functions[0].allocations:
        if not isinstance(alloc, mybir.MemoryLocationSet):
            continue
        name = alloc.memorylocations[0].name
        if alloc.kind == "ExternalInput":
            if name != partition_name:
                in_names.append(name)
        elif alloc.kind == "ExternalOutput":
            out_names.append(name)
            out_avals.append(jax.core.ShapedArray(
                tuple(alloc.tensor_shape), mybir.dt.np(alloc.dtype)))
            zero_outs.append(np.zeros(tuple(alloc.tensor_shape), mybir.dt.np(alloc.dtype)))
    n_params = len(in_names)
    in_names = in_names + out_names
    if partition_name is not None:
        in_names.append(partition_name)

    def _body(*args):
        operands = list(args)
        if partition_name is not None:
            operands.append(partition_id_tensor())
        return tuple(_bass_exec_p.bind(
            *operands, out_avals=tuple(out_avals), in_names=tuple(in_names),
            out_names=tuple(out_names), lowering_input_output_aliases=(),
            sim_require_finite=True, sim_require_nnan=True, nc=nc))

    devices = jax.devices()[:NCORES]
    mesh = Mesh(np.asarray(devices), ("core",))
    nin = n_params + len(out_names)
    fn = jax.jit(shard_map(_body, mesh=mesh, in_specs=(PartitionSpec("core"),) * nin,
                           out_specs=(PartitionSpec("core"),) * len(out_names),
                           check_rep=False), keep_unused=True)
    sharding = NamedSharding(mesh, PartitionSpec("core"))
    runner = dict(fn=fn, in_names=in_names, n_params=n_params, zero_outs=zero_outs,
                  out_names=out_names, out_avals=out_avals, sharding=sharding)
    _CACHE["runner"] = runner
    return runner


def device_inputs(inputs):
    """Pack + upload the per-core inputs; returns the device arg list."""
    import jax
    r = _get_runner()
    wmaps = _pack_weights(inputs)
    xs = _pack_x(inputs["x"])
    msks = _pack_masks()
    in_maps = [dict(wmaps, xc=xs[c], msk=msks[c]) for c in range(NCORES)]
    per_core = [[np.asarray(m[nm]) for nm in r["in_names"][:r["n_params"]]]
                for m in in_maps]
    concat_in = [np.concatenate([per_core[c][i] for c in range(NCORES)], axis=0)
                 for i in range(r["n_params"])]
    concat_zeros = [np.zeros((NCORES * z.shape[0], *z.shape[1:]), z.dtype)
                    for z in r["zero_outs"]]
    return [jax.device_put(a, r["sharding"]) for a in concat_in + concat_zeros]


def run_device(inputs, trace=False):
    """Run the conv stage on 8 cores; returns ([12,360,360] maps, None)."""
    r = _get_runner()
    dev_in = device_inputs(inputs)
    outs = r["fn"](*dev_in)
    om = np.asarray(outs[r["out_names"].index("out_maps")]).reshape(NCORES, 12, HS, WP)
    full = np.zeros((12, H, W), np.float32)
    for c in range(NCORES):
        full[:, HS * c:HS * c + HS, :] = om[c][:, :, 1:1 + W]
    return full, None


def _host_decode(full, inputs):
    """Replicate the reference decode in numpy f32 on the 12-channel maps.

    Channel layout: 0:3 hm logits, 3 iou, 4:6 ct, 6 cz, 7:10 dim, 10:12 rot
    (conv1 biases are NOT yet applied - they are added here)."""
    f = np.float32
    b = {n: np.asarray(inputs[f"b_{n}1"], f) for n in COUT}
    hm_l = full[0:3] + b["hm"][:, None, None]
    iou_m = full[3] + b["iou"][0]
    ct_m = full[4:6] + b["ct"][:, None, None]
    cz_m = full[6] + b["cz"][0]
    dm_m = full[7:10] + b["dim"][:, None, None]
    rt_m = full[10:12] + b["rot"][:, None, None]

    hw = H * W
    scores_map = (1.0 / (1.0 + np.exp(-hm_l))).astype(f)
    flat = scores_map.reshape(-1)

    # exact top-K with jax.lax.top_k tie-breaking (value desc, index asc)
    ncand = min(K + 64, flat.size)
    cand = np.argpartition(-flat, ncand - 1)[:ncand]
    cand = cand[np.lexsort((cand, -flat[cand]))]
    v = flat[cand[K - 1]]
    above = cand[flat[cand] > v]
    ties = np.nonzero(flat == v)[0]
    sel = np.concatenate([above, ties[:K - len(above)]])
    scores = flat[sel]
    inds = sel.astype(np.int32)

    labels = inds // hw
    sp = inds % hw
    ys = (sp // W).astype(f)
    xs = (sp % W).astype(f)

    ctg = ct_m.reshape(2, hw)[:, sp]
    czg = cz_m.reshape(hw)[sp]
    dmg = np.exp(dm_m.reshape(3, hw)[:, sp]).astype(f)
    rtg = rt_m.reshape(2, hw)[:, sp]
    iog = np.clip((iou_m.reshape(hw)[sp] + f(1.0)) * f(0.5), f(0.0), f(1.0))

    xs = (xs + ctg[0]) * f(STRIDE) * f(VOXEL) + f(PC_MIN_X)
    ys = (ys + ctg[1]) * f(STRIDE) * f(VOXEL) + f(PC_MIN_Y)
    ang = np.arctan2(rtg[1], rtg[0]).astype(f)
    boxes = np.stack([xs, ys, czg, dmg[0], dmg[1], dmg[2], ang], axis=-1)

    r = IOU_RECT[labels]
    scores = (np.power(scores, (f(1.0) - r)) * np.power(iog, r)).astype(f)

    pr = POST_RANGE
    in_range = (boxes[:, :3] >= pr[:3]).all(axis=-1) & (boxes[:, :3] <= pr[3:]).all(axis=-1)
    valid = (scores > f(SCORE_TH)) & in_range
    scores = np.where(valid, scores, f(0.0)).astype(f)

    order = np.argsort(-scores, kind="stable")
    boxes_s = boxes[order]
    scores_s = scores[order]
    labels_s = labels[order]
    valid_s = valid[order]

    # greedy class-agnostic BEV NMS (exact reference replication)
    x, y, dx, dy = boxes_s[:, 0], boxes_s[:, 1], boxes_s[:, 3], boxes_s[:, 4]
    x1, x2 = x - dx * f(0.5), x + dx * f(0.5)
    y1, y2 = y - dy * f(0.5), y + dy * f(0.5)
    ix = np.maximum(f(0.0), np.minimum(x2[:, None], x2[None, :]) - np.maximum(x1[:, None], x1[None, :]))
    iy = np.maximum(f(0.0), np.minimum(y2[:, None], y2[None, :]) - np.maximum(y1[:, None], y1[None, :]))
    inter = (ix * iy).astype(f)
    area = (dx * dy).astype(f)
    iou = inter / np.maximum(area[:, None] + area[None, :] - inter, f(1e-6))
    idx = np.arange(K)
    keep = valid_s.copy()
    for i in range(K):
        sup = keep[i] & (iou[i] > f(NMS_TH)) & (idx > i)
        keep &= ~sup

    return (boxes_s[None].astype(np.float32),
            (scores_s * keep.astype(f))[None].astype(np.float32),
            labels_s[None].astype(np.int32),
            keep[None])


def kernel(**inputs):
    full, _ = run_device(inputs)
    return _host_decode(full, inputs)


# revision 14
# speedup vs baseline: 1.1829x; 1.1829x over previous
"""CenterHead inference kernel for Trainium2 (8 NeuronCores, SPMD).

Strategy
--------
Spatially shard the 360-row BEV map into 8 H-shards of 45 rows. Each core
receives its own zero-padded x slab (51 rows incl. 3-row halo each side) plus
the full (tiny) weight set, and computes its shard of all 12 output channels
(hm logits x3, iou, ct x2, cz, dim x3, rot x2) with exact-fp32 matmuls:

  shared 3x3 conv (128->64) -> per branch-pair 3x3 conv (64->128, two
  branches packed into the M dim, 3x3 taps pair-packed into K via a
  row-shifted duplicate of the shared feature map) -> 3x3 conv to the
  per-branch output channels (branch pair block-diagonal in K).

Convs are computed as flat-image shift-and-accumulate matmuls over the
W-padded (362-wide) row-major layout; junk values produced at the pad
columns are re-zeroed between stages and stripped at the end.

The decode (sigmoid/top-k/gather/box math/argsort/NMS) runs on the host on
the gathered 12x360x360 maps: it is O(K^2)=250k scalar ops vs the ~70 GFLOP
conv stage, and keeping it in fp64-free numpy f32 reproduces the reference
bit-layout closely.
"""

import os

import numpy as np

B, CIN, CSH, H, W = 1, 128, 64, 360, 360
NUM_CLS = 3
K = 500
SCORE_TH = 0.1
NMS_TH = 0.7
VOXEL = 0.32
STRIDE = 1
PC_MIN_X, PC_MIN_Y = -57.6, -57.6
POST_RANGE = np.array([-61.2, -61.2, -10.0, 61.2, 61.2, 10.0], np.float32)
IOU_RECT = np.array([0.68, 0.71, 0.65], np.float32)

NCORES = 8
HS = H // NCORES           # 45 out rows per core
WP = W + 2                 # 362 padded width
XROWS = HS + 6             # 51 x rows per core (3-row halo each side)
XN = XROWS * WP            # 18462
SHROWS = HS + 4            # 49 shared rows ([-2, +2) halo)
SHN = SHROWS * WP          # 17738
HROWS = HS + 2             # 47 hidden rows ([-1, +1) halo)
HN = HROWS * WP            # 17014
ON = HS * WP               # 16290
MMC = 512                  # matmul free-dim chunk

# branch pairs: precision-critical (hm, iou) first; couts per branch
PAIRS = [("hm", "iou"), ("ct", "cz"), ("dim", "rot")]
COUT = {"hm": 3, "ct": 2, "cz": 1, "dim": 3, "rot": 2, "iou": 1}
PAIR_M = [COUT[a] + COUT[b] for a, b in PAIRS]        # [4, 3, 5]
PAIR_ROW0 = [0, 4, 7]                                 # out_maps row offsets
TAPS = [(dy, dx) for dy in range(3) for dx in range(3)]

_CACHE = {}
_MM_F32R = bool(os.environ.get("KERNEL_F32R"))  # fp32r matmuls (4x faster, reduced precision)


def _build_program(reps=1):
    import concourse.bass as bass
    import concourse.tile as tile
    from concourse import bacc, mybir

    f32 = mybir.dt.float32
    nc = bacc.Bacc(
        "TRN2",
        debug=False,
        enable_asserts=False,
        target_bir_lowering=False,
        num_devices=NCORES,
    )

    # --- DRAM I/O ---
    x_d = nc.dram_tensor("xc", [128, XN], f32, kind="ExternalInput").ap()
    msk_d = nc.dram_tensor("msk", [128, 51 + HROWS], f32, kind="ExternalInput").ap()
    wsh_d = nc.dram_tensor("wsh", [128, 9 * 64], f32, kind="ExternalInput").ap()
    ssh_d = nc.dram_tensor("ssh", [64, 1], f32, kind="ExternalInput").ap()
    bsh_d = nc.dram_tensor("bsh", [64, 1], f32, kind="ExternalInput").ap()
    w0d02_d, w0d1_d, s0_d, b0_d, w1_d = [], [], [], [], []
    for p, m in enumerate(PAIR_M):
        w0d02_d.append(nc.dram_tensor(f"w0d02_{p}", [128, 3 * 128], f32, kind="ExternalInput").ap())
        w0d1_d.append(nc.dram_tensor(f"w0d1_{p}", [64, 3 * 128], f32, kind="ExternalInput").ap())
        s0_d.append(nc.dram_tensor(f"s0_{p}", [128, 1], f32, kind="ExternalInput").ap())
        b0_d.append(nc.dram_tensor(f"b0_{p}", [128, 1], f32, kind="ExternalInput").ap())
        w1_d.append(nc.dram_tensor(f"w1_{p}", [128, 3 * 96], f32, kind="ExternalInput").ap())
    out_d = nc.dram_tensor("out_maps", [12, ON], f32, kind="ExternalOutput").ap()

    with tile.TileContext(nc) as tc:
        for _ in range(reps):
            _emit(tc, nc, bass, mybir, x_d, msk_d, wsh_d, ssh_d, bsh_d,
                  w0d02_d, w0d1_d, s0_d, b0_d, w1_d, out_d)

    nc.compile()
    return nc


def _emit(tc, nc, bass, mybir, x_d, msk_d, wsh_d, ssh_d, bsh_d,
          w0d02_d, w0d1_d, s0_d, b0_d, w1_d, out_d):
    from contextlib import ExitStack

    f32 = mybir.dt.float32
    AF = mybir.ActivationFunctionType
    mmdt = mybir.dt.float32r if _MM_F32R else f32

    def mm(out, lhsT, rhs, start, stop):
        nc.tensor.matmul(out, lhsT.bitcast(mmdt), rhs.bitcast(mmdt),
                         start=start, stop=stop)

    with ExitStack() as ctx:
        constp = ctx.enter_context(tc.tile_pool(name="const", bufs=1))

        def load_const(ap_d, shape, tag):
            t = constp.tile(shape, f32, tag=tag)
            nc.sync.dma_start(t[:], ap_d[:])
            return t

        wsh = load_const(wsh_d, [128, 9 * 64], "wsh")
        mskt = load_const(msk_d, [128, 51 + HROWS], "msk")
        ssh = load_const(ssh_d, [64, 1], "ssh")
        bsh = load_const(bsh_d, [64, 1], "bsh")
        w0d02 = [load_const(w0d02_d[p], [128, 3 * 128], f"w0d02_{p}") for p in range(3)]
        w0d1 = [load_const(w0d1_d[p], [64, 3 * 128], f"w0d1_{p}") for p in range(3)]
        s0 = [load_const(s0_d[p], [128, 1], f"s0_{p}") for p in range(3)]
        b0 = [load_const(b0_d[p], [128, 1], f"b0_{p}") for p in range(3)]
        w1 = [load_const(w1_d[p], [128, 3 * 96], f"w1_{p}") for p in range(3)]

        # shared feature map, duplicated layout:
        #   partitions 0:64  row j (of 51)   = shared local row j-1 (rows 0,50 zero)
        #   partitions 64:128 row j          = lower row j+2
        shp = ctx.enter_context(tc.tile_pool(name="shp", bufs=1))
        sh = shp.tile([128, 51 * WP], f32)

        # ---- phase A/B: x DMA + shared conv ----
        with tc.tile_pool(name="xp", bufs=1) as xp, \
             tc.tile_pool(name="psA", bufs=4, space="PSUM") as psA_pool:
            xt = xp.tile([128, XN + 2], f32)   # 1-elem guards both ends
            nc.vector.memset(xt[:, 0:1], 0.0)
            nc.vector.memset(xt[:, XN + 1:XN + 2], 0.0)
            # zero regions of sh that are never written but are read
            nc.vector.memset(sh[0:64, 0:WP], 0.0)                       # lower row 0
            nc.vector.memset(sh[0:64, 50 * WP:51 * WP], 0.0)            # lower row 50
            nc.vector.memset(sh[64:128, 48 * WP:51 * WP], 0.0)          # upper rows 48..50

            # x DMA in 6 row-chunks for overlap with compute
            row_edges = [0, 9, 18, 27, 36, 44, 51]
            for r0, r1 in zip(row_edges[:-1], row_edges[1:]):
                nc.sync.dma_start(xt[:, 1 + r0 * WP:1 + r1 * WP],
                                  x_d[:, r0 * WP:r1 * WP])

            for g0 in range(0, SHN, MMC):
                n = min(MMC, SHN - g0)
                ps = psA_pool.tile([64, MMC], f32, tag="psA")
                for t, (dy, dx) in enumerate(TAPS):
                    o = 1 + g0 + dy * WP + dx - 1
                    mm(ps[:, :n], wsh[:, 64 * t:64 * t + 64],
                       xt[:, o:o + n], start=(t == 0), stop=(t == 8))
                # lower copy: relu(ps*s+b) -> sh[0:64] at flat g0+WP
                nc.scalar.activation(sh[0:64, WP + g0:WP + g0 + n], ps[:, :n],
                                     AF.Relu, bias=bsh[:, 0:1], scale=ssh[:, 0:1])
                # upper copy: same values shifted down 2 rows (skip local row 0)
                s_off = max(0, WP - g0)
                if n > s_off:
                    nc.scalar.activation(
                        sh[64:128, g0 - WP + s_off:g0 - WP + n], ps[:, s_off:n],
                        AF.Relu, bias=bsh[:, 0:1], scale=ssh[:, 0:1])

        # re-zero the W-pad columns (flat-conv wrote junk there)
        sh3 = sh.rearrange("p (r c) -> p r c", c=WP)
        nc.vector.memset(sh3[0:64, 1:50, 0:1], 0.0)
        nc.vector.memset(sh3[0:64, 1:50, 361:362], 0.0)
        nc.vector.memset(sh3[64:128, 0:48, 0:1], 0.0)
        nc.vector.memset(sh3[64:128, 0:48, 361:362], 0.0)
        # zero the out-of-image halo rows (reference SAME-pad semantics);
        # mask is per-core input data so the SPMD program stays uniform
        msk_sh = mskt[:, 0:51].rearrange("p (r o) -> p r o", o=1).broadcast_to((128, 51, WP))
        nc.vector.tensor_mul(sh3[:, :, :], sh3[:, :, :], msk_sh)

        # ---- phase C/D: branch pairs ----
        with tc.tile_pool(name="hp", bufs=1) as hp, \
             tc.tile_pool(name="bounce", bufs=6) as bouncep:
            for p in range(3):
                m = PAIR_M[p]
                row0 = PAIR_ROW0[p]
                hid = hp.tile([128, HN + 2], f32, tag="hidden")
                nc.vector.memset(hid[:, 0:1], 0.0)
                nc.vector.memset(hid[:, HN + 1:HN + 2], 0.0)

                # conv0: shared(64) -> pair hidden(128); taps (0,dx)+(2,dx)
                # pair-packed in K via the upper copy of sh; (1,dx) K=64.
                psB_pool = tc.alloc_tile_pool(name=f"psB{p}", bufs=4, space="PSUM")
                for f0 in range(0, HN, MMC):
                    n = min(MMC, HN - f0)
                    ps = psB_pool.tile([128, MMC], f32, tag="psB")
                    for i, dx in enumerate(range(3)):
                        o = f0 + WP + dx - 1
                        mm(ps[:, :n], w0d02[p][:, 128 * dx:128 * dx + 128],
                           sh[0:128, o:o + n], start=(i == 0), stop=False)
                    for i, dx in enumerate(range(3)):
                        o = f0 + 2 * WP + dx - 1
                        mm(ps[:, :n], w0d1[p][0:64, 128 * dx:128 * dx + 128],
                           sh[0:64, o:o + n], start=False, stop=(i == 2))
                    nc.scalar.activation(hid[:, 1 + f0:1 + f0 + n], ps[:, :n],
                                         AF.Relu, bias=b0[p][:, 0:1], scale=s0[p][:, 0:1])

                hid3 = hid[:, 1:1 + HN].rearrange("p (r c) -> p r c", c=WP)
                nc.vector.memset(hid3[:, :, 0:1], 0.0)
                nc.vector.memset(hid3[:, :, 361:362], 0.0)
                msk_h = mskt[:, 51:51 + HROWS].rearrange(
                    "p (r o) -> p r o", o=1).broadcast_to((128, HROWS, WP))
                nc.vector.tensor_mul(hid3[:, :, :], hid3[:, :, :], msk_h)
                psB_pool.release()
                psC_pool = tc.alloc_tile_pool(name=f"psC{p}", bufs=2, space="PSUM")

                # conv1 via M=(3 dy)x(m) partial sums: per 4-row hidden
                # chunk, 3 dx-matmuls produce g[(dy,c), hy, x]; the dy row
                # shift is applied afterwards on DVE (2 adds per dy).
                gts = {}
                n_g = (HROWS + 3) // 4            # 12 g chunks (last has 3 rows)
                for k in range(n_g):
                    rows = min(4, HROWS - 4 * k)
                    span = rows * WP
                    gt = psC_pool.tile([96, 4 * WP], f32, tag="psC")
                    gts[k] = gt
                    for sc0 in range(0, span, MMC):
                        ns = min(MMC, span - sc0)
                        for dx in range(3):
                            o = 1 + 4 * k * WP + sc0 + dx - 1
                            mm(gt[:96, sc0:sc0 + ns], w1[p][:, 96 * dx:96 * dx + 96],
                               hid[:, o:o + ns], start=(dx == 0), stop=(dx == 2))
                    # dy-sum for the out chunk that is now fully computable
                    for ok in ([k - 1] if k > 0 else []) + ([k] if k == n_g - 1 else []):
                        o0 = 4 * ok
                        o1 = min(o0 + 4, HS)
                        if o1 <= o0:
                            continue
                        osp = (o1 - o0) * WP
                        bt = bouncep.tile([8, 4 * WP], f32, tag="bounce")
                        ga, gb = gts[ok], gts.get(ok + 1)
                        # dy=0: g rows [o0, o1) == tile ok rows [0, o1-o0)
                        nc.vector.tensor_copy(bt[:m, :osp], ga[0:m, 0:osp])
                        # dy=1: g rows [o0+1, o1+1)
                        a_rows = min(o1 + 1, 4 * ok + 4) - (o0 + 1)
                        if a_rows > 0:
                            nc.vector.tensor_add(bt[:m, 0:a_rows * WP], bt[:m, 0:a_rows * WP],
                                                 ga[32:32 + m, WP:WP + a_rows * WP])
                        b_rows = (o1 + 1) - max(o0 + 1, 4 * ok + 4)
                        if b_rows > 0:
                            nc.vector.tensor_add(bt[:m, a_rows * WP:osp], bt[:m, a_rows * WP:osp],
                                                 gb[32:32 + m, 0:b_rows * WP])
                        # dy=2: g rows [o0+2, o1+2)
                        a_rows2 = min(o1 + 2, 4 * ok + 4) - (o0 + 2)
                        if a_rows2 > 0:
                            nc.vector.tensor_add(bt[:m, 0:a_rows2 * WP], bt[:m, 0:a_rows2 * WP],
                                                 ga[64:64 + m, 2 * WP:2 * WP + a_rows2 * WP])
                        b_rows2 = (o1 + 2) - max(o0 + 2, 4 * ok + 4)
                        if b_rows2 > 0:
                            nc.vector.tensor_add(bt[:m, a_rows2 * WP:osp], bt[:m, a_rows2 * WP:osp],
                                                 gb[64:64 + m, 0:b_rows2 * WP])
                        nc.sync.dma_start(out_d[row0:row0 + m, o0 * WP:o0 * WP + osp],
                                          bt[:m, :osp])
                        gts.pop(ok - 1, None)
                psC_pool.release()


def _pack_weights(inputs):
    f = np.float32
    a = {k: np.asarray(v, f) for k, v in inputs.items()}
    maps = {}
    w_sh = a["w_shared"]  # [64,128,3,3]
    maps["wsh"] = np.ascontiguousarray(np.concatenate(
        [w_sh[:, :, dy, dx].T for dy, dx in TAPS], axis=1))
    maps["ssh"] = a["s_shared"][:, None]
    maps["bsh"] = a["b_shared"][:, None]
    for p, (A, Bn) in enumerate(PAIRS):
        wA0, wB0 = a[f"w_{A}0"], a[f"w_{Bn}0"]

        def pairM(dy, dx):
            return np.concatenate([wA0[:, :, dy, dx].T, wB0[:, :, dy, dx].T], axis=1)

        maps[f"w0d02_{p}"] = np.ascontiguousarray(np.concatenate(
            [np.concatenate([pairM(0, dx), pairM(2, dx)], axis=0) for dx in range(3)],
            axis=1))
        maps[f"w0d1_{p}"] = np.ascontiguousarray(np.concatenate(
            [pairM(1, dx) for dx in range(3)], axis=1))
        maps[f"s0_{p}"] = np.concatenate([a[f"s_{A}0"], a[f"s_{Bn}0"]])[:, None]
        maps[f"b0_{p}"] = np.concatenate([a[f"b_{A}0"], a[f"b_{Bn}0"]])[:, None]
        wA1, wB1 = a[f"w_{A}1"], a[f"w_{Bn}1"]
        cA, cB = COUT[A], COUT[Bn]
        cols = []
        for dx in range(3):
            z = np.zeros((128, 96), f)
            for dy in range(3):
                z[0:64, 32 * dy:32 * dy + cA] = wA1[:, :, dy, dx].T
                z[64:128, 32 * dy + cA:32 * dy + cA + cB] = wB1[:, :, dy, dx].T
            cols.append(z)
        maps[f"w1_{p}"] = np.ascontiguousarray(np.concatenate(cols, axis=1))
    return {k: np.ascontiguousarray(v, f) for k, v in maps.items()}


def _pack_x(x):
    xp = np.zeros((128, H + 6, WP), np.float32)
    xp[:, 3:3 + H, 1:1 + W] = np.asarray(x, np.float32)[0]
    return [np.ascontiguousarray(xp[:, HS * c:HS * c + XROWS, :].reshape(128, XN))
            for c in range(NCORES)]


def _pack_masks():
    msks = []
    for c in range(NCORES):
        m = np.zeros((128, 51 + HROWS), np.float32)
        for j in range(51):
            m[0:64, j] = 1.0 if 0 <= 45 * c + j - 3 <= 359 else 0.0
            m[64:128, j] = 1.0 if 0 <= 45 * c + j - 1 <= 359 else 0.0
        for hy in range(HROWS):
            m[:, 51 + hy] = 1.0 if 0 <= 45 * c - 1 + hy <= 359 else 0.0
        msks.append(m)
    return msks


def _get_runner(nc=None):
    """Build (once) the jitted 8-core shard_map runner for the Bass program."""
    cache = nc is None
    if cache and "runner" in _CACHE:
        return _CACHE["runner"]
    import jax
    from jax.sharding import Mesh, NamedSharding, PartitionSpec
    from jax.experimental.shard_map import shard_map
    import concourse.mybir as mybir
    from concourse.bass2jax import _bass_exec_p, partition_id_tensor

    if nc is None:
        if "nc" not in _CACHE:
            _CACHE["nc"] = _build_program()
        nc = _CACHE["nc"]

    partition_name = nc.partition_id_tensor.name if nc.partition_id_tensor else None
    in_names, out_names, out_avals, zero_outs = [], [], [], []
    for alloc in nc.m.functions[0].allocations:
        if not isinstance(alloc, mybir.MemoryLocationSet):
            continue
        name = alloc.memorylocations[0].name
        if alloc.kind == "ExternalInput":
            if name != partition_name:
                in_names.append(name)
        elif alloc.kind == "ExternalOutput":
            out_names.append(name)
            out_avals.append(jax.core.ShapedArray(
                tuple(alloc.tensor_shape), mybir.dt.np(alloc.dtype)))
            zero_outs.append(np.zeros(tuple(alloc.tensor_shape), mybir.dt.np(alloc.dtype)))
    n_params = len(in_names)
    in_names = in_names + out_names
    if partition_name is not None:
        in_names.append(partition_name)

    def _body(*args):
        operands = list(args)
        if partition_name is not None:
            operands.append(partition_id_tensor())
        return tuple(_bass_exec_p.bind(
            *operands, out_avals=tuple(out_avals), in_names=tuple(in_names),
            out_names=tuple(out_names), lowering_input_output_aliases=(),
            sim_require_finite=True, sim_require_nnan=True, nc=nc))

    devices = jax.devices()[:NCORES]
    mesh = Mesh(np.asarray(devices), ("core",))
    nin = n_params + len(out_names)
    fn = jax.jit(shard_map(_body, mesh=mesh, in_specs=(PartitionSpec("core"),) * nin,
                           out_specs=(PartitionSpec("core"),) * len(out_names),
                           check_rep=False), keep_unused=True)
    sharding = NamedSharding(mesh, PartitionSpec("core"))
    runner = dict(fn=fn, in_names=in_names, n_params=n_params, zero_outs=zero_outs,
                  out_names=out_names, out_avals=out_avals, sharding=sharding)
    if cache:
        _CACHE["runner"] = runner
    return runner


def device_inputs(inputs, r=None):
    """Pack + upload the per-core inputs; returns the device arg list."""
    import jax
    if r is None:
        r = _get_runner()
    wmaps = _pack_weights(inputs)
    xs = _pack_x(inputs["x"])
    msks = _pack_masks()
    in_maps = [dict(wmaps, xc=xs[c], msk=msks[c]) for c in range(NCORES)]
    per_core = [[np.asarray(m[nm]) for nm in r["in_names"][:r["n_params"]]]
                for m in in_maps]
    concat_in = [np.concatenate([per_core[c][i] for c in range(NCORES)], axis=0)
                 for i in range(r["n_params"])]
    concat_zeros = [np.zeros((NCORES * z.shape[0], *z.shape[1:]), z.dtype)
                    for z in r["zero_outs"]]
    return [jax.device_put(a, r["sharding"]) for a in concat_in + concat_zeros]


def run_device(inputs, trace=False):
    """Run the conv stage on 8 cores; returns ([12,360,360] maps, None)."""
    r = _get_runner()
    dev_in = device_inputs(inputs)
    outs = r["fn"](*dev_in)
    om = np.asarray(outs[r["out_names"].index("out_maps")]).reshape(NCORES, 12, HS, WP)
    full = np.zeros((12, H, W), np.float32)
    for c in range(NCORES):
        full[:, HS * c:HS * c + HS, :] = om[c][:, :, 1:1 + W]
    return full, None


def _host_decode(full, inputs):
    """Replicate the reference decode in numpy f32 on the 12-channel maps.

    Channel layout: 0:3 hm logits, 3 iou, 4:6 ct, 6 cz, 7:10 dim, 10:12 rot
    (conv1 biases are NOT yet applied - they are added here)."""
    f = np.float32
    b = {n: np.asarray(inputs[f"b_{n}1"], f) for n in COUT}
    hm_l = full[0:3] + b["hm"][:, None, None]
    iou_m = full[3] + b["iou"][0]
    ct_m = full[4:6] + b["ct"][:, None, None]
    cz_m = full[6] + b["cz"][0]
    dm_m = full[7:10] + b["dim"][:, None, None]
    rt_m = full[10:12] + b["rot"][:, None, None]

    hw = H * W
    scores_map = (1.0 / (1.0 + np.exp(-hm_l))).astype(f)
    flat = scores_map.reshape(-1)

    # exact top-K with jax.lax.top_k tie-breaking (value desc, index asc)
    ncand = min(K + 64, flat.size)
    cand = np.argpartition(-flat, ncand - 1)[:ncand]
    cand = cand[np.lexsort((cand, -flat[cand]))]
    v = flat[cand[K - 1]]
    above = cand[flat[cand] > v]
    ties = np.nonzero(flat == v)[0]
    sel = np.concatenate([above, ties[:K - len(above)]])
    scores = flat[sel]
    inds = sel.astype(np.int32)

    labels = inds // hw
    sp = inds % hw
    ys = (sp // W).astype(f)
    xs = (sp % W).astype(f)

    ctg = ct_m.reshape(2, hw)[:, sp]
    czg = cz_m.reshape(hw)[sp]
    dmg = np.exp(dm_m.reshape(3, hw)[:, sp]).astype(f)
    rtg = rt_m.reshape(2, hw)[:, sp]
    iog = np.clip((iou_m.reshape(hw)[sp] + f(1.0)) * f(0.5), f(0.0), f(1.0))

    xs = (xs + ctg[0]) * f(STRIDE) * f(VOXEL) + f(PC_MIN_X)
    ys = (ys + ctg[1]) * f(STRIDE) * f(VOXEL) + f(PC_MIN_Y)
    ang = np.arctan2(rtg[1], rtg[0]).astype(f)
    boxes = np.stack([xs, ys, czg, dmg[0], dmg[1], dmg[2], ang], axis=-1)

    r = IOU_RECT[labels]
    scores = (np.power(scores, (f(1.0) - r)) * np.power(iog, r)).astype(f)

    pr = POST_RANGE
    in_range = (boxes[:, :3] >= pr[:3]).all(axis=-1) & (boxes[:, :3] <= pr[3:]).all(axis=-1)
    valid = (scores > f(SCORE_TH)) & in_range
    scores = np.where(valid, scores, f(0.0)).astype(f)

    order = np.argsort(-scores, kind="stable")
    boxes_s = boxes[order]
    scores_s = scores[order]
    labels_s = labels[order]
    valid_s = valid[order]

    # greedy class-agnostic BEV NMS (exact reference replication)
    x, y, dx, dy = boxes_s[:, 0], boxes_s[:, 1], boxes_s[:, 3], boxes_s[:, 4]
    x1, x2 = x - dx * f(0.5), x + dx * f(0.5)
    y1, y2 = y - dy * f(0.5), y + dy * f(0.5)
    ix = np.maximum(f(0.0), np.minimum(x2[:, None], x2[None, :]) - np.maximum(x1[:, None], x1[None, :]))
    iy = np.maximum(f(0.0), np.minimum(y2[:, None], y2[None, :]) - np.maximum(y1[:, None], y1[None, :]))
    inter = (ix * iy).astype(f)
    area = (dx * dy).astype(f)
    iou = inter / np.maximum(area[:, None] + area[None, :] - inter, f(1e-6))
    idx = np.arange(K)
    keep = valid_s.copy()
    for i in range(K):
        sup = keep[i] & (iou[i] > f(NMS_TH)) & (idx > i)
        keep &= ~sup

    return (boxes_s[None].astype(np.float32),
            (scores_s * keep.astype(f))[None].astype(np.float32),
            labels_s[None].astype(np.int32),
            keep[None])


def kernel(**inputs):
    full, _ = run_device(inputs)
    return _host_decode(full, inputs)


# revision 19
# speedup vs baseline: 1.3466x; 1.1384x over previous
"""CenterHead inference kernel for Trainium2 (8 NeuronCores, SPMD).

Strategy
--------
Spatially shard the 360-row BEV map into 8 H-shards of 45 rows. Each core
receives its own zero-padded x slab (51 rows incl. 3-row halo each side) plus
the full (tiny) weight set, and computes its shard of all 12 output channels
(hm logits x3, iou, ct x2, cz, dim x3, rot x2) with exact-fp32 matmuls:

  shared 3x3 conv (128->64) -> per branch-pair 3x3 conv (64->128, two
  branches packed into the M dim, 3x3 taps pair-packed into K via a
  row-shifted duplicate of the shared feature map) -> 3x3 conv to the
  per-branch output channels (branch pair block-diagonal in K).

Convs are computed as flat-image shift-and-accumulate matmuls over the
W-padded (362-wide) row-major layout; junk values produced at the pad
columns are re-zeroed between stages and stripped at the end.

The decode (sigmoid/top-k/gather/box math/argsort/NMS) runs on the host on
the gathered 12x360x360 maps: it is O(K^2)=250k scalar ops vs the ~70 GFLOP
conv stage, and keeping it in fp64-free numpy f32 reproduces the reference
bit-layout closely.
"""

import os

import numpy as np

B, CIN, CSH, H, W = 1, 128, 64, 360, 360
NUM_CLS = 3
K = 500
SCORE_TH = 0.1
NMS_TH = 0.7
VOXEL = 0.32
STRIDE = 1
PC_MIN_X, PC_MIN_Y = -57.6, -57.6
POST_RANGE = np.array([-61.2, -61.2, -10.0, 61.2, 61.2, 10.0], np.float32)
IOU_RECT = np.array([0.68, 0.71, 0.65], np.float32)

NCORES = 8
HS = H // NCORES           # 45 out rows per core
WP = W + 2                 # 362 padded width
XROWS = HS + 6             # 51 x rows per core (3-row halo each side)
XN = XROWS * WP            # 18462
SHROWS = HS + 4            # 49 shared rows ([-2, +2) halo)
SHN = SHROWS * WP          # 17738
HROWS = HS + 2             # 47 hidden rows ([-1, +1) halo)
HN = HROWS * WP            # 17014
ON = HS * WP               # 16290
MMC = 512                  # matmul free-dim chunk

# branch pairs: precision-critical (hm, iou) first; couts per branch
PAIRS = [("hm", "iou"), ("ct", "cz"), ("dim", "rot")]
COUT = {"hm": 3, "ct": 2, "cz": 1, "dim": 3, "rot": 2, "iou": 1}
PAIR_M = [COUT[a] + COUT[b] for a, b in PAIRS]        # [4, 3, 5]
PAIR_ROW0 = [0, 4, 7]                                 # out_maps row offsets
TAPS = [(dy, dx) for dy in range(3) for dx in range(3)]

_CACHE = {}
_MM_F32R = bool(os.environ.get("KERNEL_F32R"))  # fp32r matmuls (4x faster, reduced precision)


def _build_program(reps=1):
    import concourse.bass as bass
    import concourse.tile as tile
    from concourse import bacc, mybir

    f32 = mybir.dt.float32
    nc = bacc.Bacc(
        "TRN2",
        debug=False,
        enable_asserts=False,
        target_bir_lowering=False,
        num_devices=NCORES,
    )

    # --- DRAM I/O ---
    x_d = nc.dram_tensor("xc", [128, XN], f32, kind="ExternalInput").ap()
    msk_d = nc.dram_tensor("msk", [128, 51 + HROWS], f32, kind="ExternalInput").ap()
    wsh_d = nc.dram_tensor("wsh", [128, 9 * 64], f32, kind="ExternalInput").ap()
    ssh_d = nc.dram_tensor("ssh", [64, 1], f32, kind="ExternalInput").ap()
    bsh_d = nc.dram_tensor("bsh", [64, 1], f32, kind="ExternalInput").ap()
    w0d02_d, w0d1_d, s0_d, b0_d, w1_d = [], [], [], [], []
    for p, m in enumerate(PAIR_M):
        w0d02_d.append(nc.dram_tensor(f"w0d02_{p}", [128, 3 * 128], f32, kind="ExternalInput").ap())
        w0d1_d.append(nc.dram_tensor(f"w0d1_{p}", [64, 3 * 128], f32, kind="ExternalInput").ap())
        s0_d.append(nc.dram_tensor(f"s0_{p}", [128, 1], f32, kind="ExternalInput").ap())
        b0_d.append(nc.dram_tensor(f"b0_{p}", [128, 1], f32, kind="ExternalInput").ap())
        w1_d.append(nc.dram_tensor(f"w1_{p}", [128, 3 * 96], f32, kind="ExternalInput").ap())
    out_d = nc.dram_tensor("out_maps", [12, ON], f32, kind="ExternalOutput").ap()

    with tile.TileContext(nc) as tc:
        for _ in range(reps):
            _emit(tc, nc, bass, mybir, x_d, msk_d, wsh_d, ssh_d, bsh_d,
                  w0d02_d, w0d1_d, s0_d, b0_d, w1_d, out_d)

    nc.compile()
    return nc


def _emit(tc, nc, bass, mybir, x_d, msk_d, wsh_d, ssh_d, bsh_d,
          w0d02_d, w0d1_d, s0_d, b0_d, w1_d, out_d):
    from contextlib import ExitStack

    f32 = mybir.dt.float32
    AF = mybir.ActivationFunctionType
    mmdt = mybir.dt.float32r if _MM_F32R else f32

    def mm(out, lhsT, rhs, start, stop):
        nc.tensor.matmul(out, lhsT.bitcast(mmdt), rhs.bitcast(mmdt),
                         start=start, stop=stop)

    with ExitStack() as ctx:
        constp = ctx.enter_context(tc.tile_pool(name="const", bufs=1))

        def load_const(ap_d, shape, tag):
            t = constp.tile(shape, f32, tag=tag)
            nc.sync.dma_start(t[:], ap_d[:])
            return t

        wsh = load_const(wsh_d, [128, 9 * 64], "wsh")
        mskt = load_const(msk_d, [128, 51 + HROWS], "msk")
        ssh = load_const(ssh_d, [64, 1], "ssh")
        bsh = load_const(bsh_d, [64, 1], "bsh")

        # shared feature map, duplicated layout:
        #   partitions 0:64  row j (of 51)   = shared local row j-1 (rows 0,50 zero)
        #   partitions 64:128 row j          = lower row j+2
        shp = ctx.enter_context(tc.tile_pool(name="shp", bufs=1))
        sh = shp.tile([128, 51 * WP], f32)

        # ---- phase A/B: x DMA + shared conv ----
        with tc.tile_pool(name="xp", bufs=1) as xp, \
             tc.tile_pool(name="psA", bufs=4, space="PSUM") as psA_pool:
            xt = xp.tile([128, XN + 2], f32)   # 1-elem guards both ends
            nc.vector.memset(xt[:, 0:1], 0.0)
            nc.vector.memset(xt[:, XN + 1:XN + 2], 0.0)
            # zero regions of sh that are never written but are read
            nc.vector.memset(sh[0:64, 0:WP], 0.0)                       # lower row 0
            nc.vector.memset(sh[0:64, 50 * WP:51 * WP], 0.0)            # lower row 50
            nc.vector.memset(sh[64:128, 48 * WP:51 * WP], 0.0)          # upper rows 48..50

            # x DMA in row-chunks for overlap with compute; the branch
            # weights are loaded after the first x chunks so the shared conv
            # can start as early as possible.
            row_edges = [0, 5, 11, 18, 26, 35, 44, 51]
            for r0, r1 in zip(row_edges[:-1], row_edges[1:]):
                nc.sync.dma_start(xt[:, 1 + r0 * WP:1 + r1 * WP],
                                  x_d[:, r0 * WP:r1 * WP])
            w0d02 = [load_const(w0d02_d[p], [128, 3 * 128], f"w0d02_{p}") for p in range(3)]
            w0d1 = [load_const(w0d1_d[p], [64, 3 * 128], f"w0d1_{p}") for p in range(3)]
            s0 = [load_const(s0_d[p], [128, 1], f"s0_{p}") for p in range(3)]
            b0 = [load_const(b0_d[p], [128, 1], f"b0_{p}") for p in range(3)]
            w1 = [load_const(w1_d[p], [128, 3 * 96], f"w1_{p}") for p in range(3)]

            for g0 in range(0, SHN, MMC):
                n = min(MMC, SHN - g0)
                ps = psA_pool.tile([64, MMC], f32, tag="psA")
                for t, (dy, dx) in enumerate(TAPS):
                    o = 1 + g0 + dy * WP + dx - 1
                    mm(ps[:, :n], wsh[:, 64 * t:64 * t + 64],
                       xt[:, o:o + n], start=(t == 0), stop=(t == 8))
                # lower copy: relu(ps*s+b) -> sh[0:64] at flat g0+WP
                nc.scalar.activation(sh[0:64, WP + g0:WP + g0 + n], ps[:, :n],
                                     AF.Relu, bias=bsh[:, 0:1], scale=ssh[:, 0:1])
                # upper copy: same values shifted down 2 rows (skip local row 0)
                s_off = max(0, WP - g0)
                if n > s_off:
                    nc.scalar.activation(
                        sh[64:128, g0 - WP + s_off:g0 - WP + n], ps[:, s_off:n],
                        AF.Relu, bias=bsh[:, 0:1], scale=ssh[:, 0:1])

        # re-zero the W-pad columns (flat-conv wrote junk there) and the
        # out-of-image halo rows (reference SAME-pad semantics; the mask is
        # per-core input data so the SPMD program stays uniform). Chunked by
        # row blocks so DVE overlaps the tail of the shared conv instead of
        # gating all of conv0 on one full-tensor op.
        sh3 = sh.rearrange("p (r c) -> p r c", c=WP)
        msk_sh = mskt[:, 0:51].rearrange("p (r o) -> p r o", o=1)
        for r0 in range(0, 51, 6):
            r1 = min(r0 + 6, 51)
            lo0, lo1 = max(r0, 1), min(r1, 50)
            if lo1 > lo0:
                nc.vector.memset(sh3[0:64, lo0:lo1, 0:1], 0.0)
                nc.vector.memset(sh3[0:64, lo0:lo1, 361:362], 0.0)
            up1 = min(r1, 48)
            if up1 > r0:
                nc.vector.memset(sh3[64:128, r0:up1, 0:1], 0.0)
                nc.vector.memset(sh3[64:128, r0:up1, 361:362], 0.0)
            nc.vector.tensor_mul(sh3[:, r0:r1, :], sh3[:, r0:r1, :],
                                 msk_sh[:, r0:r1, :].broadcast_to((128, r1 - r0, WP)))

        # ---- phase C/D: branch pairs ----
        with tc.tile_pool(name="hp", bufs=1) as hp, \
             tc.tile_pool(name="bounce", bufs=6) as bouncep:
            for p in range(3):
                m = PAIR_M[p]
                row0 = PAIR_ROW0[p]
                hid = hp.tile([128, HN + 2], f32, tag="hidden")
                nc.vector.memset(hid[:, 0:1], 0.0)
                nc.vector.memset(hid[:, HN + 1:HN + 2], 0.0)

                # conv0: shared(64) -> pair hidden(128); taps (0,dx)+(2,dx)
                # pair-packed in K via the upper copy of sh; (1,dx) K=64.
                psB_pool = tc.alloc_tile_pool(name=f"psB{p}", bufs=4, space="PSUM")
                for f0 in range(0, HN, MMC):
                    n = min(MMC, HN - f0)
                    ps = psB_pool.tile([128, MMC], f32, tag="psB")
                    for i, dx in enumerate(range(3)):
                        o = f0 + WP + dx - 1
                        mm(ps[:, :n], w0d02[p][:, 128 * dx:128 * dx + 128],
                           sh[0:128, o:o + n], start=(i == 0), stop=False)
                    for i, dx in enumerate(range(3)):
                        o = f0 + 2 * WP + dx - 1
                        mm(ps[:, :n], w0d1[p][0:64, 128 * dx:128 * dx + 128],
                           sh[0:64, o:o + n], start=False, stop=(i == 2))
                    nc.scalar.activation(hid[:, 1 + f0:1 + f0 + n], ps[:, :n],
                                         AF.Relu, bias=b0[p][:, 0:1], scale=s0[p][:, 0:1])

                hid3 = hid[:, 1:1 + HN].rearrange("p (r c) -> p r c", c=WP)
                msk_h = mskt[:, 51:51 + HROWS].rearrange("p (r o) -> p r o", o=1)
                for r0 in range(0, HROWS, 6):
                    r1 = min(r0 + 6, HROWS)
                    nc.vector.memset(hid3[:, r0:r1, 0:1], 0.0)
                    nc.vector.memset(hid3[:, r0:r1, 361:362], 0.0)
                    nc.vector.tensor_mul(hid3[:, r0:r1, :], hid3[:, r0:r1, :],
                                         msk_h[:, r0:r1, :].broadcast_to((128, r1 - r0, WP)))
                psB_pool.release()
                psC_pool = tc.alloc_tile_pool(name=f"psC{p}", bufs=2, space="PSUM")

                # conv1 via M=(3 dy)x(m) partial sums: per 4-row hidden
                # chunk, 3 dx-matmuls produce g[(dy,c), hy, x] with the dy
                # blocks padded to 32-partition boundaries (engine reads must
                # start at mod-32 partitions); the dy row shift is applied
                # afterwards on DVE (2 adds per dy).
                gts = {}
                n_g = (HROWS + 3) // 4            # 12 g chunks (last has 3 rows)
                for k in range(n_g):
                    rows = min(4, HROWS - 4 * k)
                    span = rows * WP
                    gt = psC_pool.tile([96, 4 * WP], f32, tag="psC")
                    gts[k] = gt
                    for sc0 in range(0, span, MMC):
                        ns = min(MMC, span - sc0)
                        for dx in range(3):
                            o = 1 + 4 * k * WP + sc0 + dx - 1
                            mm(gt[:96, sc0:sc0 + ns], w1[p][:, 96 * dx:96 * dx + 96],
                               hid[:, o:o + ns], start=(dx == 0), stop=(dx == 2))
                    # dy-sum for the out chunk that is now fully computable
                    for ok in ([k - 1] if k > 0 else []) + ([k] if k == n_g - 1 else []):
                        o0 = 4 * ok
                        o1 = min(o0 + 4, HS)
                        if o1 <= o0:
                            continue
                        osp = (o1 - o0) * WP
                        bt = bouncep.tile([8, 4 * WP], f32, tag="bounce")
                        ga, gb = gts[ok], gts.get(ok + 1)
                        # dy=0: g rows [o0, o1) == tile ok rows [0, o1-o0)
                        nc.vector.tensor_copy(bt[:m, :osp], ga[0:m, 0:osp])
                        # dy=1: g rows [o0+1, o1+1)
                        a_rows = min(o1 + 1, 4 * ok + 4) - (o0 + 1)
                        if a_rows > 0:
                            nc.vector.tensor_add(bt[:m, 0:a_rows * WP], bt[:m, 0:a_rows * WP],
                                                 ga[32:32 + m, WP:WP + a_rows * WP])
                        b_rows = (o1 + 1) - max(o0 + 1, 4 * ok + 4)
                        if b_rows > 0:
                            nc.vector.tensor_add(bt[:m, a_rows * WP:osp], bt[:m, a_rows * WP:osp],
                                                 gb[32:32 + m, 0:b_rows * WP])
                        # dy=2: g rows [o0+2, o1+2)
                        a_rows2 = min(o1 + 2, 4 * ok + 4) - (o0 + 2)
                        if a_rows2 > 0:
                            nc.vector.tensor_add(bt[:m, 0:a_rows2 * WP], bt[:m, 0:a_rows2 * WP],
                                                 ga[64:64 + m, 2 * WP:2 * WP + a_rows2 * WP])
                        b_rows2 = (o1 + 2) - max(o0 + 2, 4 * ok + 4)
                        if b_rows2 > 0:
                            nc.vector.tensor_add(bt[:m, a_rows2 * WP:osp], bt[:m, a_rows2 * WP:osp],
                                                 gb[64:64 + m, 0:b_rows2 * WP])
                        nc.sync.dma_start(out_d[row0:row0 + m, o0 * WP:o0 * WP + osp],
                                          bt[:m, :osp])
                        gts.pop(ok - 1, None)
                psC_pool.release()


def _pack_weights(inputs):
    f = np.float32
    a = {k: np.asarray(v, f) for k, v in inputs.items()}
    maps = {}
    w_sh = a["w_shared"]  # [64,128,3,3]
    maps["wsh"] = np.ascontiguousarray(np.concatenate(
        [w_sh[:, :, dy, dx].T for dy, dx in TAPS], axis=1))
    maps["ssh"] = a["s_shared"][:, None]
    maps["bsh"] = a["b_shared"][:, None]
    for p, (A, Bn) in enumerate(PAIRS):
        wA0, wB0 = a[f"w_{A}0"], a[f"w_{Bn}0"]

        def pairM(dy, dx):
            return np.concatenate([wA0[:, :, dy, dx].T, wB0[:, :, dy, dx].T], axis=1)

        maps[f"w0d02_{p}"] = np.ascontiguousarray(np.concatenate(
            [np.concatenate([pairM(0, dx), pairM(2, dx)], axis=0) for dx in range(3)],
            axis=1))
        maps[f"w0d1_{p}"] = np.ascontiguousarray(np.concatenate(
            [pairM(1, dx) for dx in range(3)], axis=1))
        maps[f"s0_{p}"] = np.concatenate([a[f"s_{A}0"], a[f"s_{Bn}0"]])[:, None]
        maps[f"b0_{p}"] = np.concatenate([a[f"b_{A}0"], a[f"b_{Bn}0"]])[:, None]
        wA1, wB1 = a[f"w_{A}1"], a[f"w_{Bn}1"]
        cA, cB = COUT[A], COUT[Bn]
        cols = []
        for dx in range(3):
            z = np.zeros((128, 96), f)
            for dy in range(3):
                z[0:64, 32 * dy:32 * dy + cA] = wA1[:, :, dy, dx].T
                z[64:128, 32 * dy + cA:32 * dy + cA + cB] = wB1[:, :, dy, dx].T
            cols.append(z)
        maps[f"w1_{p}"] = np.ascontiguousarray(np.concatenate(cols, axis=1))
    return {k: np.ascontiguousarray(v, f) for k, v in maps.items()}


def _pack_x(x):
    xp = np.zeros((128, H + 6, WP), np.float32)
    xp[:, 3:3 + H, 1:1 + W] = np.asarray(x, np.float32)[0]
    return [np.ascontiguousarray(xp[:, HS * c:HS * c + XROWS, :].reshape(128, XN))
            for c in range(NCORES)]


def _pack_masks():
    msks = []
    for c in range(NCORES):
        m = np.zeros((128, 51 + HROWS), np.float32)
        for j in range(51):
            m[0:64, j] = 1.0 if 0 <= 45 * c + j - 3 <= 359 else 0.0
            m[64:128, j] = 1.0 if 0 <= 45 * c + j - 1 <= 359 else 0.0
        for hy in range(HROWS):
            m[:, 51 + hy] = 1.0 if 0 <= 45 * c - 1 + hy <= 359 else 0.0
        msks.append(m)
    return msks


def _get_runner(nc=None):
    """Build (once) the jitted 8-core shard_map runner for the Bass program."""
    cache = nc is None
    if cache and "runner" in _CACHE:
        return _CACHE["runner"]
    import jax
    from jax.sharding import Mesh, NamedSharding, PartitionSpec
    from jax.experimental.shard_map import shard_map
    import concourse.mybir as mybir
    from concourse.bass2jax import _bass_exec_p, partition_id_tensor

    if nc is None:
        if "nc" not in _CACHE:
            _CACHE["nc"] = _build_program()
        nc = _CACHE["nc"]

    partition_name = nc.partition_id_tensor.name if nc.partition_id_tensor else None
    in_names, out_names, out_avals, zero_outs = [], [], [], []
    for alloc in nc.m.functions[0].allocations:
        if not isinstance(alloc, mybir.MemoryLocationSet):
            continue
        name = alloc.memorylocations[0].name
        if alloc.kind == "ExternalInput":
            if name != partition_name:
                in_names.append(name)
        elif alloc.kind == "ExternalOutput":
            out_names.append(name)
            out_avals.append(jax.core.ShapedArray(
                tuple(alloc.tensor_shape), mybir.dt.np(alloc.dtype)))
            zero_outs.append(np.zeros(tuple(alloc.tensor_shape), mybir.dt.np(alloc.dtype)))
    n_params = len(in_names)
    in_names = in_names + out_names
    if partition_name is not None:
        in_names.append(partition_name)

    def _body(*args):
        operands = list(args)
        if partition_name is not None:
            operands.append(partition_id_tensor())
        return tuple(_bass_exec_p.bind(
            *operands, out_avals=tuple(out_avals), in_names=tuple(in_names),
            out_names=tuple(out_names), lowering_input_output_aliases=(),
            sim_require_finite=True, sim_require_nnan=True, nc=nc))

    devices = jax.devices()[:NCORES]
    mesh = Mesh(np.asarray(devices), ("core",))
    nin = n_params + len(out_names)
    fn = jax.jit(shard_map(_body, mesh=mesh, in_specs=(PartitionSpec("core"),) * nin,
                           out_specs=(PartitionSpec("core"),) * len(out_names),
                           check_rep=False), keep_unused=True)
    sharding = NamedSharding(mesh, PartitionSpec("core"))
    runner = dict(fn=fn, in_names=in_names, n_params=n_params, zero_outs=zero_outs,
                  out_names=out_names, out_avals=out_avals, sharding=sharding)
    if cache:
        _CACHE["runner"] = runner
    return runner


def device_inputs(inputs, r=None):
    """Pack + upload the per-core inputs; returns the device arg list."""
    import jax
    if r is None:
        r = _get_runner()
    wmaps = _pack_weights(inputs)
    xs = _pack_x(inputs["x"])
    msks = _pack_masks()
    in_maps = [dict(wmaps, xc=xs[c], msk=msks[c]) for c in range(NCORES)]
    per_core = [[np.asarray(m[nm]) for nm in r["in_names"][:r["n_params"]]]
                for m in in_maps]
    concat_in = [np.concatenate([per_core[c][i] for c in range(NCORES)], axis=0)
                 for i in range(r["n_params"])]
    concat_zeros = [np.zeros((NCORES * z.shape[0], *z.shape[1:]), z.dtype)
                    for z in r["zero_outs"]]
    return [jax.device_put(a, r["sharding"]) for a in concat_in + concat_zeros]


def run_device(inputs, trace=False):
    """Run the conv stage on 8 cores; returns ([12,360,360] maps, None)."""
    r = _get_runner()
    dev_in = device_inputs(inputs)
    outs = r["fn"](*dev_in)
    om = np.asarray(outs[r["out_names"].index("out_maps")]).reshape(NCORES, 12, HS, WP)
    full = np.zeros((12, H, W), np.float32)
    for c in range(NCORES):
        full[:, HS * c:HS * c + HS, :] = om[c][:, :, 1:1 + W]
    return full, None


def _host_decode(full, inputs):
    """Replicate the reference decode in numpy f32 on the 12-channel maps.

    Channel layout: 0:3 hm logits, 3 iou, 4:6 ct, 6 cz, 7:10 dim, 10:12 rot
    (conv1 biases are NOT yet applied - they are added here)."""
    f = np.float32
    b = {n: np.asarray(inputs[f"b_{n}1"], f) for n in COUT}
    hm_l = full[0:3] + b["hm"][:, None, None]
    iou_m = full[3] + b["iou"][0]
    ct_m = full[4:6] + b["ct"][:, None, None]
    cz_m = full[6] + b["cz"][0]
    dm_m = full[7:10] + b["dim"][:, None, None]
    rt_m = full[10:12] + b["rot"][:, None, None]

    hw = H * W
    scores_map = (1.0 / (1.0 + np.exp(-hm_l))).astype(f)
    flat = scores_map.reshape(-1)

    # exact top-K with jax.lax.top_k tie-breaking (value desc, index asc)
    ncand = min(K + 64, flat.size)
    cand = np.argpartition(-flat, ncand - 1)[:ncand]
    cand = cand[np.lexsort((cand, -flat[cand]))]
    v = flat[cand[K - 1]]
    above = cand[flat[cand] > v]
    ties = np.nonzero(flat == v)[0]
    sel = np.concatenate([above, ties[:K - len(above)]])
    scores = flat[sel]
    inds = sel.astype(np.int32)

    labels = inds // hw
    sp = inds % hw
    ys = (sp // W).astype(f)
    xs = (sp % W).astype(f)

    ctg = ct_m.reshape(2, hw)[:, sp]
    czg = cz_m.reshape(hw)[sp]
    dmg = np.exp(dm_m.reshape(3, hw)[:, sp]).astype(f)
    rtg = rt_m.reshape(2, hw)[:, sp]
    iog = np.clip((iou_m.reshape(hw)[sp] + f(1.0)) * f(0.5), f(0.0), f(1.0))

    xs = (xs + ctg[0]) * f(STRIDE) * f(VOXEL) + f(PC_MIN_X)
    ys = (ys + ctg[1]) * f(STRIDE) * f(VOXEL) + f(PC_MIN_Y)
    ang = np.arctan2(rtg[1], rtg[0]).astype(f)
    boxes = np.stack([xs, ys, czg, dmg[0], dmg[1], dmg[2], ang], axis=-1)

    r = IOU_RECT[labels]
    scores = (np.power(scores, (f(1.0) - r)) * np.power(iog, r)).astype(f)

    pr = POST_RANGE
    in_range = (boxes[:, :3] >= pr[:3]).all(axis=-1) & (boxes[:, :3] <= pr[3:]).all(axis=-1)
    valid = (scores > f(SCORE_TH)) & in_range
    scores = np.where(valid, scores, f(0.0)).astype(f)

    order = np.argsort(-scores, kind="stable")
    boxes_s = boxes[order]
    scores_s = scores[order]
    labels_s = labels[order]
    valid_s = valid[order]

    # greedy class-agnostic BEV NMS (exact reference replication)
    x, y, dx, dy = boxes_s[:, 0], boxes_s[:, 1], boxes_s[:, 3], boxes_s[:, 4]
    x1, x2 = x - dx * f(0.5), x + dx * f(0.5)
    y1, y2 = y - dy * f(0.5), y + dy * f(0.5)
    ix = np.maximum(f(0.0), np.minimum(x2[:, None], x2[None, :]) - np.maximum(x1[:, None], x1[None, :]))
    iy = np.maximum(f(0.0), np.minimum(y2[:, None], y2[None, :]) - np.maximum(y1[:, None], y1[None, :]))
    inter = (ix * iy).astype(f)
    area = (dx * dy).astype(f)
    iou = inter / np.maximum(area[:, None] + area[None, :] - inter, f(1e-6))
    idx = np.arange(K)
    keep = valid_s.copy()
    for i in range(K):
        sup = keep[i] & (iou[i] > f(NMS_TH)) & (idx > i)
        keep &= ~sup

    return (boxes_s[None].astype(np.float32),
            (scores_s * keep.astype(f))[None].astype(np.float32),
            labels_s[None].astype(np.int32),
            keep[None])


def kernel(**inputs):
    full, _ = run_device(inputs)
    return _host_decode(full, inputs)
